# revision 22
# baseline (speedup 1.0000x reference)
"""EMAStats segment-reduce kernel for 8 Trainium2 NeuronCores (Bass/Tile).

Problem: given logits [B, K], target [B], running (mean, var, count) [K]:
  own[i]     = logits[i, target[i]]
  per class c: n_c = #{i: t_i=c}, s_c = sum own, q_c = sum own^2
  batch_mean = s/n, batch_var = q/n - batch_mean^2
  EMA update with decay 0.1 (first update uses batch stats); classes with
  n_c = 0 keep their buffers.

Strategy (data-parallel over B, 8 cores, BS = 16384 rows/core):
  1. dma_gather (SWDGE) fetches, per row, the 256-byte block of its logits
     row containing column target[i] (16 gathers x 1024 rows, int16 idx).
     A dummy warmup gather at kernel start forces the Pool engine's gather
     ucode library load off the critical path.  Gather indices derive from
     a PE transpose of the naturally-loaded target tile plus
     partition-group fold copies (stream_shuffle keeps DVE partition
     offsets quadrant-aligned).
  2. fp16 histogram: v = sum_e G16*.E (E = 64-wide one-hot, prebuilt), PE
     matmuls use fp16 one-hot weights (FWL) with one 64-column rhs per
     128-token tile: [count, v16, r16*4096, v16^2] x 16 B-groups, where
     r16 = fp16(4096*(v - fp16(v))) is the scaled sum residual that
     restores ~f32 accuracy for batch_mean (the scale dodges fp16
     subnormal flush; it is divided back out after PSUM).
  3. All-reduce of the [128, 48] partials via direct SBUF->SBUF
     remote_dma_broadcast to all 8 cores (slot j on the receiver holds the
     partial of core (self XOR j)) + a local tree reduction — avoids the
     ~37us collective_compute path.
  4. EMA update redundantly on every core.
"""

import numpy as np

import concourse.bacc as bacc
import concourse.bass as bass
import concourse.mybir as mybir
import concourse.tile as tile
from concourse.bass_utils import run_bass_kernel_spmd

B, K = 131072, 2048
NCORES = 8
BS = B // NCORES  # 16384 rows per core
P = 128
NG = 16  # B-groups (class & 15)
NSUB = 16  # sub-shards for int16 gather indices
SUBR = BS // NSUB  # 1024 rows per sub-shard
BLK = 64  # f32 elements per gathered block (256 bytes)
NSTAT = 4  # count, v16, r16, v16^2
RSCALE = 4096.0  # residual scale (keeps r16 out of fp16 subnormals)
EMA_DECAY = 0.1
EPS = 1e-12
USE_RDMA_ALLREDUCE = False

F32 = mybir.dt.float32
F16 = mybir.dt.float16
I32 = mybir.dt.int32
I16 = mybir.dt.int16
U8 = mybir.dt.uint8

OP = mybir.AluOpType
AF = mybir.ActivationFunctionType


def build_program() -> bass.Bass:
    nc = bacc.Bacc(
        trn_type="TRN2", num_devices=NCORES, debug=False, num_swdge_queues=4
    )

    lg = nc.dram_tensor("logits", [BS * K, 1], F32, kind="ExternalInput")
    tgt = nc.dram_tensor("target", [BS], I32, kind="ExternalInput")
    mean_in = nc.dram_tensor("mean", [K], F32, kind="ExternalInput")
    var_in = nc.dram_tensor("var", [K], F32, kind="ExternalInput")
    cnt_in = nc.dram_tensor("count", [K], I32, kind="ExternalInput")

    new_mean = nc.dram_tensor("new_mean", [K], F32, kind="ExternalOutput")
    new_var = nc.dram_tensor("new_var", [K], F32, kind="ExternalOutput")
    new_count = nc.dram_tensor("new_count", [K], I32, kind="ExternalOutput")

    if not USE_RDMA_ALLREDUCE:
        cc_in = nc.dram_tensor("cc_in", [P, 48], F32)
        cc_out = nc.dram_tensor("cc_out", [P, 48], F32)

    # constants baked into the NEFF
    ident_c = nc.inline_tensor(np.eye(P, dtype=np.float32), name="ident_c")
    # aff[p, q] = ((q % 8)*128 + p)*32 : affine part of the gather index in
    # tl space (token (p, q) -> in-gather slot (q%8)*128 + p).
    qq = np.arange(P, dtype=np.int32)[None, :]
    pp = np.arange(P, dtype=np.int32)[:, None]
    aff_np = ((qq % 8) * 128 + pp) * 32
    afft_c = nc.inline_tensor(aff_np, name="afft_c")
    iota128_c = nc.inline_tensor(
        np.broadcast_to(np.arange(P, dtype=np.float16), (P, P)).copy(),
        name="iota128_c",
    )
    iota64_c = nc.inline_tensor(
        np.broadcast_to(np.arange(BLK, dtype=np.float32), (P, BLK)).copy(),
        name="iota64_c",
    )
    iota16_c = nc.inline_tensor(
        np.broadcast_to(np.arange(NG, dtype=np.float16), (P, NG)).copy(),
        name="iota16_c",
    )
    dummy_idx_c = nc.inline_tensor(
        np.zeros((P, 8), dtype=np.int16), name="dummy_idx_c"
    )

    with tile.TileContext(nc) as tc:
        with (
            tc.tile_pool(name="sb", bufs=1) as sb,
            tc.tile_pool(name="ps", bufs=1, space="PSUM") as ps,
        ):
            lgb = lg[:].rearrange("(r e) x -> r (e x)", e=BLK)  # [BS*K/64, 64]

            # ---- warmup: a tiny dummy gather pulls the Pool engine's
            # gather-ucode library load + register setup off the critical
            # path while the real gather indices are still being built.
            dummy_idx = sb.tile([P, 8], I16)
            nc.scalar.dma_start(out=dummy_idx[:], in_=dummy_idx_c[:, :])
            scratch = sb.tile([P, BLK], F32)
            nc.gpsimd.dma_gather(
                scratch[:].rearrange("p (q e) -> p q e", e=BLK),
                lgb[0 : SUBR * (K // BLK), :],
                dummy_idx[:],
                P,
                P,
                elem_size=BLK,
                queue_num=0,
            )
            nidx_reg = nc.gpsimd.to_reg(SUBR)

            # ---- early constant + input loads (spread over HWDGE queues)
            ident = sb.tile([P, P], F32)
            nc.scalar.dma_start(out=ident[:], in_=ident_c[:, :])
            afft = sb.tile([P, P], I32)
            nc.scalar.dma_start(out=afft[:], in_=afft_c[:, :])
            iota128 = sb.tile([P, P], F16)
            nc.scalar.dma_start(out=iota128[:], in_=iota128_c[:, :])
            iota64 = sb.tile([P, BLK], F32)
            nc.scalar.dma_start(out=iota64[:], in_=iota64_c[:, :])
            iota16 = sb.tile([P, NG], F16)
            nc.scalar.dma_start(out=iota16[:], in_=iota16_c[:, :])
            m_t = sb.tile([P, NG], F32)
            nc.scalar.dma_start(out=m_t[:], in_=mean_in[:].rearrange("(p c) -> p c", p=P))
            va_t = sb.tile([P, NG], F32)
            nc.scalar.dma_start(out=va_t[:], in_=var_in[:].rearrange("(p c) -> p c", p=P))
            c_t = sb.tile([P, NG], I32)
            nc.scalar.dma_start(out=c_t[:], in_=cnt_in[:].rearrange("(p c) -> p c", p=P))

            # ---- target: natural contiguous load, PE transpose -> tl
            tnat = sb.tile([P, P], I32)
            nc.sync.dma_start(out=tnat[:], in_=tgt[:].rearrange("(p f) -> p f", p=P))
            tnatf = sb.tile([P, P], F32)
            nc.vector.tensor_copy(out=tnatf[:], in_=tnat[:])
            ptr = ps.tile([P, P], F32, name="ptr")
            nc.tensor.transpose(out=ptr[:], in_=tnatf[:], identity=ident[:])
            tl = sb.tile([P, P], I32)
            nc.vector.tensor_copy(out=tl[:], in_=ptr[:])

            # ---- gather indices: idxval = aff | (tl >> 6), fold to the
            # 16-partition wrap, cast int16, replicate to 128 partitions.
            blk_t = sb.tile([P, P], I32)
            nc.vector.tensor_scalar(
                out=blk_t[:], in0=tl[:], scalar1=6, scalar2=None,
                op0=OP.arith_shift_right,
            )
            idxval = sb.tile([P, P], I32)
            nc.vector.tensor_tensor(
                out=idxval[:], in0=blk_t[:], in1=afft[:], op=OP.bitwise_or
            )
            # DVE partition offsets must be quadrant-aligned (0/32/64/96):
            # rotate each 32-partition quadrant by 16 so the odd 16-row
            # groups also sit at aligned starts, then fold with 8 copies.
            idxvr = sb.tile([P, P], I32)
            nc.vector.stream_shuffle(
                out=idxvr[:], in_=idxval[:], mask=[(i + 16) % 32 for i in range(32)]
            )
            idx16 = sb.tile([P, BS // 16], I16)
            i16v = idx16[:16, :].rearrange("b (g c a) -> b g c a", g=16, c=8)
            iv3 = idxval[:].rearrange("p (g c) -> p g c", c=8)
            ivr3 = idxvr[:].rearrange("p (g c) -> p g c", c=8)
            for alpha in range(8):
                src = iv3 if alpha % 2 == 0 else ivr3
                base = 16 * alpha if alpha % 2 == 0 else 16 * (alpha - 1)
                # even folds on DVE, odd folds on the (idle) Pool engine
                eng = nc.vector if alpha % 2 == 0 else nc.gpsimd
                eng.tensor_copy(
                    out=i16v[:, :, :, alpha],
                    in_=src[base : base + 16, :, :],
                )
            for step in range(3):
                w = 16 << step
                nc.sync.dma_start(out=idx16[w : 2 * w, :], in_=idx16[:w, :])

            # ---- the 16 gathers (descriptor gen serializes on Pool)
            g_t = sb.tile([P, P * BLK], F32)
            g3 = g_t[:].rearrange("p (q e) -> p q e", e=BLK)
            for g in range(NSUB):
                nc.gpsimd.dma_gather(
                    g3[:, 8 * g : 8 * (g + 1), :],
                    lgb[g * SUBR * (K // BLK) : (g + 1) * SUBR * (K // BLK), :],
                    idx16[:, (SUBR // 16) * g : (SUBR // 16) * (g + 1)],
                    SUBR,
                    nidx_reg,
                    elem_size=BLK,
                    queue_num=g % 4,
                )

            # ---- all-reduce plumbing (descriptors prepped early on Pool;
            # the data dependency on `st` defers to the trigger)
            st = sb.tile([P, 48], F32)
            allg = sb.tile([P, 8 * 48], F32)
            if USE_RDMA_ALLREDUCE:
                rsem = nc.alloc_semaphore("rdma_remote")
                lsem = nc.alloc_semaphore("rdma_local")
                for j in range(1, NCORES):
                    rdests = [None] * 8
                    rdests[j] = (0, j)  # slot j <=> Δtpb j (D2D bit matches)
                    nc.gpsimd.remote_dma_broadcast(
                        allg[:, 48 * j : 48 * (j + 1)],
                        st[:],
                        rsem,
                        lsem,
                        rdests=rdests,
                        queue_num=0,
                    )

            # ---- class decomposition (int32, then fp16 casts)
            a_t = sb.tile([P, P], I32)
            b_t = sb.tile([P, P], I32)
            lowb = sb.tile([P, P], I32)
            nc.vector.tensor_scalar(
                out=a_t[:], in0=tl[:], scalar1=4, scalar2=None,
                op0=OP.arith_shift_right,
            )
            nc.vector.tensor_scalar(
                out=b_t[:], in0=tl[:], scalar1=15, scalar2=None,
                op0=OP.bitwise_and,
            )
            nc.vector.tensor_scalar(
                out=lowb[:], in0=tl[:], scalar1=BLK - 1, scalar2=None,
                op0=OP.bitwise_and,
            )
            a16 = sb.tile([P, P], F16)
            nc.vector.tensor_copy(out=a16[:], in_=a_t[:])
            b16 = sb.tile([P, P], F16)
            nc.vector.tensor_copy(out=b16[:], in_=b_t[:])

            # ---- persistent tiles
            # E is f32 so the extraction multiply sees the FULL-precision
            # gathered values: v must be exact f32 for the r16 residual to
            # be nonzero (an fp16 G copy would make v fp16-exact, r = 0).
            e_t = sb.tile([P, P * BLK], F32)
            e3 = e_t[:].rearrange("p (q e) -> p q e", e=BLK)
            lowbf = sb.tile([P, P], F32)
            nc.vector.tensor_copy(out=lowbf[:], in_=lowb[:])
            iota64f = iota64
            # vmall[p, c, s, g]: s = 0 count mask, 1 v16, 2 r16, 3 v16^2
            vmall = sb.tile([P, P * NSTAT * NG], F16)
            vm4 = vmall[:].rearrange("p (c s g) -> p c s g", s=NSTAT, g=NG)
            vmflat = vmall[:].rearrange("p (c m) -> p c m", m=NSTAT * NG)
            vstk = sb.tile([P, P * 3], F16)
            vs3 = vstk[:].rearrange("p (c s) -> p c s", s=3)
            v_t = sb.tile([P, P], F32)
            up_t = sb.tile([P, P], F32)

            oh8s = [None] * NSUB

            def build_oh8_pair(j):
                # one [128, 2048] is_eq builds the A one-hot for TWO tile
                # groups (fewer DVE instruction overheads)
                oh16 = sb.tile([P, 16 * P], F16, name=f"oh16_{j}")
                nc.vector.tensor_tensor(
                    out=oh16[:].rearrange("p (c a) -> p c a", a=P),
                    in0=a16[:, 8 * j : 8 * (j + 2)][:, :, None].to_broadcast([P, 16, P]),
                    in1=iota128[:, None, :].to_broadcast([P, 16, P]),
                    op=OP.is_equal,
                )
                oh8s[j] = (oh16, 0)
                oh8s[j + 1] = (oh16, 8 * P)

            def build_e_chunk(h):
                cs = slice(32 * h, 32 * (h + 1))
                nc.vector.tensor_tensor(
                    out=e3[:, cs, :],
                    in0=lowbf[:, cs][:, :, None].to_broadcast([P, 32, BLK]),
                    in1=iota64f[:, None, :].to_broadcast([P, 32, BLK]),
                    op=OP.is_equal,
                )

            # B-group one-hot: built once into the count plane (DVE), then
            # replicated to the three value planes on the ACT engine.
            nc.vector.tensor_tensor(
                out=vm4[:, :, 0, :],
                in0=b16[:, :, None].to_broadcast([P, P, NG]),
                in1=iota16[:, None, :].to_broadcast([P, P, NG]),
                op=OP.is_equal,
            )
            for s in (1, 2, 3):
                nc.scalar.copy(out=vm4[:, :, s, :], in_=vm4[:, :, 0, :])

            build_e_chunk(0)
            build_oh8_pair(0)

            pstats = ps.tile([P, NSTAT * NG], F32)

            def process_gather(g, eng):
                # eng: DVE for early gathers; the Pool engine takes the last
                # ones (it sits idle once gather descriptor-gen finishes,
                # while DVE is the backlogged engine gating the PE chain).
                cs = slice(8 * g, 8 * (g + 1))
                # extract: prod = G * E f32 (in place into E), v = sum_e
                eng.tensor_tensor(
                    out=e3[:, cs, :], in0=g3[:, cs, :], in1=e3[:, cs, :],
                    op=OP.mult,
                )
                # free-axis reduce is DVE-only (Pool reduces partitions)
                nc.vector.tensor_reduce(
                    out=v_t[:, cs], in_=e3[:, cs, :],
                    axis=mybir.AxisListType.X, op=OP.add,
                )
                # value stack: v16; r16 = fp16(RSCALE*(v - up(v16))); v16^2
                nc.scalar.copy(out=vs3[:, cs, 0], in_=v_t[:, cs])
                nc.scalar.mul(out=up_t[:, cs], in_=vs3[:, cs, 0], mul=RSCALE)
                nc.scalar.activation(
                    out=vs3[:, cs, 2], in_=vs3[:, cs, 0], func=AF.Square
                )
                nc.vector.scalar_tensor_tensor(
                    out=vs3[:, cs, 1], in0=v_t[:, cs], scalar=RSCALE,
                    in1=up_t[:, cs], op0=OP.mult, op1=OP.subtract,
                )
                # value planes *= vstack (broadcast over the 16 B-groups)
                eng.tensor_tensor(
                    out=vm4[:, cs, 1:4, :],
                    in0=vm4[:, cs, 1:4, :],
                    in1=vs3[:, cs, :][:, :, :, None].to_broadcast([P, 8, 3, NG]),
                    op=OP.mult,
                )
                # histogram matmuls for these 8 token columns
                for c in range(8 * g, 8 * (g + 1)):
                    j, cc = divmod(c, 8)
                    oh_t, base = oh8s[j]
                    nc.tensor.matmul(
                        out=pstats[:],
                        lhsT=oh_t[:, base + P * cc : base + P * (cc + 1)],
                        rhs=vmflat[:, c, :],
                        start=(c == 0),
                        stop=(c == P - 1),
                    )

            for g in range(NSUB):
                if g % 4 == 1 and g < 12:
                    build_e_chunk(g // 4 + 1)
                if g + 1 < NSUB and oh8s[g + 1] is None:
                    build_oh8_pair(g + 1)
                process_gather(g, nc.gpsimd if g >= 12 else nc.vector)

            # first-update mask depends only on the preloaded count buffer:
            # compute it before the collective, off the critical path
            first_t = sb.tile([P, NG], U8, name="first_t")
            nc.vector.tensor_scalar(
                out=first_t[:], in0=c_t[:], scalar1=0, scalar2=None,
                op0=OP.is_equal,
            )

            # ---- fold partials [n | s_hi + s_lo/RSCALE | q] -> [128, 48]
            nc.vector.tensor_copy(out=st[:, 0:16], in_=pstats[:, 0:16])
            nc.vector.tensor_copy(out=st[:, 16:32], in_=pstats[:, 16:32])
            nc.vector.scalar_tensor_tensor(
                out=st[:, 16:32], in0=pstats[:, 32:48], scalar=1.0 / RSCALE,
                in1=st[:, 16:32], op0=OP.mult, op1=OP.add,
            )
            nc.vector.tensor_copy(out=st[:, 32:48], in_=pstats[:, 48:64])

            # ---- all-reduce across the 8 cores
            if USE_RDMA_ALLREDUCE:
                nc.gpsimd.trigger_dma(count=None)
                nc.vector.tensor_copy(out=allg[:, 0:48], in_=st[:])
                a2 = allg[:].rearrange("p (h x) -> p h x", h=2)
                red1 = sb.tile([P, 4 * 48], F32)
                # the remote-sem wait is attached AFTER TileContext exits:
                # the single-core scheduling sim cannot satisfy a semaphore
                # that only remote cores increment (deadlock otherwise)
                rdma_wait_ins = nc.vector.tensor_tensor(
                    out=red1[:], in0=a2[:, 0, :], in1=a2[:, 1, :], op=OP.add
                )
                r2 = red1[:].rearrange("p (h x) -> p h x", h=2)
                red2 = sb.tile([P, 2 * 48], F32)
                nc.vector.tensor_tensor(
                    out=red2[:], in0=r2[:, 0, :], in1=r2[:, 1, :], op=OP.add
                )
                r3 = red2[:].rearrange("p (h x) -> p h x", h=2)
                stg = sb.tile([P, 48], F32)
                nc.vector.tensor_tensor(
                    out=stg[:], in0=r3[:, 0, :], in1=r3[:, 1, :], op=OP.add
                )
            else:
                nc.sync.dma_start(out=cc_in[:, :], in_=st[:])
                nc.gpsimd.collective_compute(
                    "AllReduce",
                    OP.add,
                    replica_groups=[list(range(NCORES))],
                    ins=[cc_in.ap().opt()],
                    outs=[cc_out.ap().opt()],
                )
                stg = sb.tile([P, 48], F32)
                nc.sync.dma_start(out=stg[:], in_=cc_out[:, :])

            # ---- EMA update on [128, 16] tiles (class = p*16 + g)
            n_t = stg[:, 0:16]
            s_t = stg[:, 16:32]
            q_t = stg[:, 32:48]

            _tid = [0]

            def t16f(dtype=F32):
                _tid[0] += 1
                return sb.tile([P, NG], dtype, name=f"t16_{_tid[0]}")

            ns_t, rn_t, bm_t, bv_t = t16f(), t16f(), t16f(), t16f()
            nc.vector.tensor_scalar_max(out=ns_t[:], in0=n_t, scalar1=1.0)
            nc.vector.reciprocal(out=rn_t[:], in_=ns_t[:])
            nc.vector.tensor_tensor(out=bm_t[:], in0=s_t, in1=rn_t[:], op=OP.mult)
            qn_t, bm2_t = t16f(), t16f()
            nc.vector.tensor_tensor(out=qn_t[:], in0=q_t, in1=rn_t[:], op=OP.mult)
            nc.vector.tensor_tensor(out=bm2_t[:], in0=bm_t[:], in1=bm_t[:], op=OP.mult)
            nc.vector.tensor_tensor(
                out=bv_t[:], in0=qn_t[:], in1=bm2_t[:], op=OP.subtract
            )

            has_t = t16f(U8)
            nc.vector.tensor_scalar(
                out=has_t[:], in0=n_t, scalar1=0.0, scalar2=None, op0=OP.is_gt
            )

            d_t, em_t, ev_t = t16f(), t16f(), t16f()
            nc.vector.tensor_tensor(out=d_t[:], in0=bm_t[:], in1=m_t[:], op=OP.subtract)
            nc.vector.scalar_tensor_tensor(
                out=em_t[:], in0=d_t[:], scalar=EMA_DECAY, in1=m_t[:],
                op0=OP.mult, op1=OP.add,
            )
            nc.vector.tensor_tensor(
                out=d_t[:], in0=bv_t[:], in1=va_t[:], op=OP.subtract
            )
            nc.vector.scalar_tensor_tensor(
                out=ev_t[:], in0=d_t[:], scalar=EMA_DECAY, in1=va_t[:],
                op0=OP.mult, op1=OP.add,
            )

            cm_t, cv_t = t16f(), t16f()
            nc.vector.select(out=cm_t[:], mask=first_t[:], on_true=bm_t[:], on_false=em_t[:])
            nc.vector.select(out=cv_t[:], mask=first_t[:], on_true=bv_t[:], on_false=ev_t[:])
            nc.vector.tensor_scalar_max(out=cv_t[:], in0=cv_t[:], scalar1=EPS)

            nm_t, nv_t = t16f(), t16f()
            nc.vector.select(out=nm_t[:], mask=has_t[:], on_true=cm_t[:], on_false=m_t[:])
            nc.vector.select(out=nv_t[:], mask=has_t[:], on_true=cv_t[:], on_false=va_t[:])
            ni_t, ncnt_t = t16f(I32), t16f(I32)
            nc.vector.tensor_copy(out=ni_t[:], in_=n_t)
            nc.vector.tensor_tensor(out=ncnt_t[:], in0=c_t[:], in1=ni_t[:], op=OP.add)

            nc.sync.dma_start(
                out=new_mean[:].rearrange("(p c) -> p c", p=P), in_=nm_t[:]
            )
            nc.scalar.dma_start(
                out=new_var[:].rearrange("(p c) -> p c", p=P), in_=nv_t[:]
            )
            nc.sync.dma_start(
                out=new_count[:].rearrange("(p c) -> p c", p=P), in_=ncnt_t[:]
            )

    if USE_RDMA_ALLREDUCE:
        # Post-scheduling: require all 7 peers' partials to have landed
        # (each peer's broadcast bumps our remote sem by 16/8 = 2).
        rdma_wait_ins.wait_op(
            rsem, (NCORES - 1) * (16 // 8), "sem-ge", check=False
        )

    nc.compile()
    return nc


def make_in_maps(logits, target, mean, var, count):
    """Shard the full inputs into per-core input maps."""
    logits = np.ascontiguousarray(np.asarray(logits, dtype=np.float32))
    target = np.asarray(target).astype(np.int32)
    mean = np.asarray(mean, dtype=np.float32)
    var = np.asarray(var, dtype=np.float32)
    count_i32 = np.asarray(count).astype(np.int32)

    in_maps = []
    for m in range(NCORES):
        rows = slice(m * BS, (m + 1) * BS)
        in_maps.append(
            {
                "logits": logits[rows].reshape(BS * K, 1),
                "target": target[rows],
                "mean": mean,
                "var": var,
                "count": count_i32,
            }
        )
    return in_maps


_NC_CACHE = None


def kernel(logits, target, mean, var, count):
    global _NC_CACHE
    if _NC_CACHE is None:
        _NC_CACHE = build_program()
    nc = _NC_CACHE

    in_maps = make_in_maps(logits, target, mean, var, count)
    res = run_bass_kernel_spmd(nc, in_maps, list(range(NCORES)))
    out = res.results[0]

    count_dtype = np.asarray(count).dtype
    return (
        out["new_mean"].reshape(K).astype(np.float32),
        out["new_var"].reshape(K).astype(np.float32),
        out["new_count"].reshape(K).astype(count_dtype),
    )


# revision 24
# speedup vs baseline: 1.1489x; 1.1489x over previous
"""EMAStats segment-reduce kernel for 8 Trainium2 NeuronCores (Bass/Tile).

Problem: given logits [B, K], target [B], running (mean, var, count) [K]:
  own[i]     = logits[i, target[i]]
  per class c: n_c = #{i: t_i=c}, s_c = sum own, q_c = sum own^2
  batch_mean = s/n, batch_var = q/n - batch_mean^2
  EMA update with decay 0.1 (first update uses batch stats); classes with
  n_c = 0 keep their buffers.

Strategy (data-parallel over B, 8 cores, BS = 16384 rows/core):
  1. dma_gather (SWDGE) fetches, per row, the 256-byte block of its logits
     row containing column target[i] (16 gathers x 1024 rows, int16 idx).
     A dummy warmup gather at kernel start forces the Pool engine's gather
     ucode library load off the critical path.  Gather indices derive from
     a PE transpose of the naturally-loaded target tile plus
     partition-group fold copies (stream_shuffle keeps DVE partition
     offsets quadrant-aligned).
  2. fp16 histogram: v = sum_e G16*.E (E = 64-wide one-hot, prebuilt), PE
     matmuls use fp16 one-hot weights (FWL) with one 64-column rhs per
     128-token tile: [count, v16, r16*4096, v16^2] x 16 B-groups, where
     r16 = fp16(4096*(v - fp16(v))) is the scaled sum residual that
     restores ~f32 accuracy for batch_mean (the scale dodges fp16
     subnormal flush; it is divided back out after PSUM).
  3. All-reduce of the [128, 48] partials via direct SBUF->SBUF
     remote_dma_broadcast to all 8 cores (slot j on the receiver holds the
     partial of core (self XOR j)) + a local tree reduction — avoids the
     ~37us collective_compute path.
  4. EMA update redundantly on every core.
"""

import numpy as np

import concourse.bacc as bacc
import concourse.bass as bass
import concourse.mybir as mybir
import concourse.tile as tile
from concourse.bass_utils import run_bass_kernel_spmd

B, K = 131072, 2048
NCORES = 8
BS = B // NCORES  # 16384 rows per core
P = 128
NG = 16  # B-groups (class & 15)
NSUB = 16  # sub-shards for int16 gather indices
SUBR = BS // NSUB  # 1024 rows per sub-shard
BLK = 64  # f32 elements per gathered block (256 bytes)
NSTAT = 4  # count, v16, r16, v16^2
RSCALE = 4096.0  # residual scale (keeps r16 out of fp16 subnormals)
EMA_DECAY = 0.1
EPS = 1e-12
USE_RDMA_ALLREDUCE = False

F32 = mybir.dt.float32
F16 = mybir.dt.float16
I32 = mybir.dt.int32
I16 = mybir.dt.int16
U8 = mybir.dt.uint8

OP = mybir.AluOpType
AF = mybir.ActivationFunctionType


def build_program() -> bass.Bass:
    nc = bacc.Bacc(
        trn_type="TRN2", num_devices=NCORES, debug=False, num_swdge_queues=4
    )

    lg = nc.dram_tensor("logits", [BS * K, 1], F32, kind="ExternalInput")
    tgt = nc.dram_tensor("target", [BS], I32, kind="ExternalInput")
    mean_in = nc.dram_tensor("mean", [K], F32, kind="ExternalInput")
    var_in = nc.dram_tensor("var", [K], F32, kind="ExternalInput")
    cnt_in = nc.dram_tensor("count", [K], I32, kind="ExternalInput")

    new_mean = nc.dram_tensor("new_mean", [K], F32, kind="ExternalOutput")
    new_var = nc.dram_tensor("new_var", [K], F32, kind="ExternalOutput")
    new_count = nc.dram_tensor("new_count", [K], I32, kind="ExternalOutput")

    if not USE_RDMA_ALLREDUCE:
        cc_in = nc.dram_tensor("cc_in", [P, 48], F32)
        cc_out = nc.dram_tensor("cc_out", [P, 48], F32)

    # constants baked into the NEFF
    ident_c = nc.inline_tensor(np.eye(P, dtype=np.float32), name="ident_c")
    # aff[p, q] = ((q % 8)*128 + p)*32 : affine part of the gather index in
    # tl space (token (p, q) -> in-gather slot (q%8)*128 + p).
    qq = np.arange(P, dtype=np.int32)[None, :]
    pp = np.arange(P, dtype=np.int32)[:, None]
    aff_np = ((qq % 8) * 128 + pp) * 32
    afft_c = nc.inline_tensor(aff_np, name="afft_c")
    iota128_c = nc.inline_tensor(
        np.broadcast_to(np.arange(P, dtype=np.float16), (P, P)).copy(),
        name="iota128_c",
    )
    iota64_c = nc.inline_tensor(
        np.broadcast_to(np.arange(BLK, dtype=np.float32), (P, BLK)).copy(),
        name="iota64_c",
    )
    iota16_c = nc.inline_tensor(
        np.broadcast_to(np.arange(NG, dtype=np.float16), (P, NG)).copy(),
        name="iota16_c",
    )
    dummy_idx_c = nc.inline_tensor(
        np.zeros((P, 8), dtype=np.int16), name="dummy_idx_c"
    )

    with tile.TileContext(nc) as tc:
        with (
            tc.tile_pool(name="sb", bufs=1) as sb,
            tc.tile_pool(name="ps", bufs=1, space="PSUM") as ps,
        ):
            lgb = lg[:].rearrange("(r e) x -> r (e x)", e=BLK)  # [BS*K/64, 64]

            # ---- warmup: a tiny dummy gather pulls the Pool engine's
            # gather-ucode library load + register setup off the critical
            # path while the real gather indices are still being built.
            dummy_idx = sb.tile([P, 8], I16)
            nc.scalar.dma_start(out=dummy_idx[:], in_=dummy_idx_c[:, :])
            scratch = sb.tile([P, BLK], F32)
            nc.gpsimd.dma_gather(
                scratch[:].rearrange("p (q e) -> p q e", e=BLK),
                lgb[0 : SUBR * (K // BLK), :],
                dummy_idx[:],
                P,
                P,
                elem_size=BLK,
                queue_num=0,
            )
            nidx_reg = nc.gpsimd.to_reg(SUBR)

            # ---- early constant + input loads (spread over HWDGE queues)
            ident = sb.tile([P, P], F32)
            nc.scalar.dma_start(out=ident[:], in_=ident_c[:, :])
            afft = sb.tile([P, P], I32)
            nc.scalar.dma_start(out=afft[:], in_=afft_c[:, :])
            iota128 = sb.tile([P, P], F16)
            nc.scalar.dma_start(out=iota128[:], in_=iota128_c[:, :])
            iota64 = sb.tile([P, BLK], F32)
            nc.scalar.dma_start(out=iota64[:], in_=iota64_c[:, :])
            iota16 = sb.tile([P, NG], F16)
            nc.scalar.dma_start(out=iota16[:], in_=iota16_c[:, :])
            m_t = sb.tile([P, NG], F32)
            nc.scalar.dma_start(out=m_t[:], in_=mean_in[:].rearrange("(p c) -> p c", p=P))
            va_t = sb.tile([P, NG], F32)
            nc.scalar.dma_start(out=va_t[:], in_=var_in[:].rearrange("(p c) -> p c", p=P))
            c_t = sb.tile([P, NG], I32)
            nc.scalar.dma_start(out=c_t[:], in_=cnt_in[:].rearrange("(p c) -> p c", p=P))

            # ---- target: natural contiguous load, PE transpose -> tl
            tnat = sb.tile([P, P], I32)
            nc.sync.dma_start(out=tnat[:], in_=tgt[:].rearrange("(p f) -> p f", p=P))
            tnatf = sb.tile([P, P], F32)
            nc.vector.tensor_copy(out=tnatf[:], in_=tnat[:])
            ptr = ps.tile([P, P], F32, name="ptr")
            nc.tensor.transpose(out=ptr[:], in_=tnatf[:], identity=ident[:])
            tl = sb.tile([P, P], I32)
            nc.vector.tensor_copy(out=tl[:], in_=ptr[:])

            # ---- gather indices: idxval = aff | (tl >> 6), fold to the
            # 16-partition wrap, cast int16, replicate to 128 partitions.
            blk_t = sb.tile([P, P], I32)
            nc.vector.tensor_scalar(
                out=blk_t[:], in0=tl[:], scalar1=6, scalar2=None,
                op0=OP.arith_shift_right,
            )
            idxval = sb.tile([P, P], I32)
            nc.vector.tensor_tensor(
                out=idxval[:], in0=blk_t[:], in1=afft[:], op=OP.bitwise_or
            )
            # DVE partition offsets must be quadrant-aligned (0/32/64/96):
            # rotate each 32-partition quadrant by 16 so the odd 16-row
            # groups also sit at aligned starts, then fold with 8 copies.
            idxvr = sb.tile([P, P], I32)
            nc.vector.stream_shuffle(
                out=idxvr[:], in_=idxval[:], mask=[(i + 16) % 32 for i in range(32)]
            )
            idx16 = sb.tile([P, BS // 16], I16)
            i16v = idx16[:16, :].rearrange("b (g c a) -> b g c a", g=16, c=8)
            iv3 = idxval[:].rearrange("p (g c) -> p g c", c=8)
            ivr3 = idxvr[:].rearrange("p (g c) -> p g c", c=8)
            for alpha in range(8):
                src = iv3 if alpha % 2 == 0 else ivr3
                base = 16 * alpha if alpha % 2 == 0 else 16 * (alpha - 1)
                nc.vector.tensor_copy(
                    out=i16v[:, :, :, alpha],
                    in_=src[base : base + 16, :, :],
                )
            for step in range(3):
                w = 16 << step
                nc.sync.dma_start(out=idx16[w : 2 * w, :], in_=idx16[:w, :])

            # ---- the 16 gathers (descriptor gen serializes on Pool)
            g_t = sb.tile([P, P * BLK], F32)
            g3 = g_t[:].rearrange("p (q e) -> p q e", e=BLK)
            for g in range(NSUB):
                nc.gpsimd.dma_gather(
                    g3[:, 8 * g : 8 * (g + 1), :],
                    lgb[g * SUBR * (K // BLK) : (g + 1) * SUBR * (K // BLK), :],
                    idx16[:, (SUBR // 16) * g : (SUBR // 16) * (g + 1)],
                    SUBR,
                    nidx_reg,
                    elem_size=BLK,
                    queue_num=g % 4,
                )

            # ---- all-reduce plumbing (descriptors prepped early on Pool;
            # the data dependency on `st` defers to the trigger)
            st = sb.tile([P, 48], F32)
            allg = sb.tile([P, 8 * 48], F32)
            if USE_RDMA_ALLREDUCE:
                rsem = nc.alloc_semaphore("rdma_remote")
                lsem = nc.alloc_semaphore("rdma_local")
                for j in range(1, NCORES):
                    rdests = [None] * 8
                    rdests[j] = (0, j)  # slot j <=> Δtpb j (D2D bit matches)
                    nc.gpsimd.remote_dma_broadcast(
                        allg[:, 48 * j : 48 * (j + 1)],
                        st[:],
                        rsem,
                        lsem,
                        rdests=rdests,
                        queue_num=0,
                    )

            # ---- class decomposition (int32, then fp16 casts)
            a_t = sb.tile([P, P], I32)
            b_t = sb.tile([P, P], I32)
            lowb = sb.tile([P, P], I32)
            nc.vector.tensor_scalar(
                out=a_t[:], in0=tl[:], scalar1=4, scalar2=None,
                op0=OP.arith_shift_right,
            )
            nc.vector.tensor_scalar(
                out=b_t[:], in0=tl[:], scalar1=15, scalar2=None,
                op0=OP.bitwise_and,
            )
            nc.vector.tensor_scalar(
                out=lowb[:], in0=tl[:], scalar1=BLK - 1, scalar2=None,
                op0=OP.bitwise_and,
            )
            a16 = sb.tile([P, P], F16)
            nc.vector.tensor_copy(out=a16[:], in_=a_t[:])
            b16 = sb.tile([P, P], F16)
            nc.vector.tensor_copy(out=b16[:], in_=b_t[:])

            # ---- persistent tiles
            # E is f32 so the extraction multiply sees the FULL-precision
            # gathered values: v must be exact f32 for the r16 residual to
            # be nonzero (an fp16 G copy would make v fp16-exact, r = 0).
            e_t = sb.tile([P, P * BLK], F32)
            e3 = e_t[:].rearrange("p (q e) -> p q e", e=BLK)
            lowbf = sb.tile([P, P], F32)
            nc.vector.tensor_copy(out=lowbf[:], in_=lowb[:])
            iota64f = iota64
            # vmall[p, c, s, g]: s = 0 count mask, 1 v16, 2 r16, 3 v16^2
            vmall = sb.tile([P, P * NSTAT * NG], F16)
            vm4 = vmall[:].rearrange("p (c s g) -> p c s g", s=NSTAT, g=NG)
            vmflat = vmall[:].rearrange("p (c m) -> p c m", m=NSTAT * NG)
            vstk = sb.tile([P, P * 3], F16)
            vs3 = vstk[:].rearrange("p (c s) -> p c s", s=3)
            v_t = sb.tile([P, P], F32)
            up_t = sb.tile([P, P], F32)

            oh8s = [None] * NSUB

            def build_oh8_pair(j):
                # one [128, 2048] is_eq builds the A one-hot for TWO tile
                # groups (fewer DVE instruction overheads)
                oh16 = sb.tile([P, 16 * P], F16, name=f"oh16_{j}")
                nc.vector.tensor_tensor(
                    out=oh16[:].rearrange("p (c a) -> p c a", a=P),
                    in0=a16[:, 8 * j : 8 * (j + 2)][:, :, None].to_broadcast([P, 16, P]),
                    in1=iota128[:, None, :].to_broadcast([P, 16, P]),
                    op=OP.is_equal,
                )
                oh8s[j] = (oh16, 0)
                oh8s[j + 1] = (oh16, 8 * P)

            def build_e_chunk(h):
                cs = slice(32 * h, 32 * (h + 1))
                nc.vector.tensor_tensor(
                    out=e3[:, cs, :],
                    in0=lowbf[:, cs][:, :, None].to_broadcast([P, 32, BLK]),
                    in1=iota64f[:, None, :].to_broadcast([P, 32, BLK]),
                    op=OP.is_equal,
                )

            # B-group one-hot: built once into the count plane (DVE), then
            # replicated to the three value planes on the ACT engine.
            nc.vector.tensor_tensor(
                out=vm4[:, :, 0, :],
                in0=b16[:, :, None].to_broadcast([P, P, NG]),
                in1=iota16[:, None, :].to_broadcast([P, P, NG]),
                op=OP.is_equal,
            )
            for s in (1, 2, 3):
                nc.scalar.copy(out=vm4[:, :, s, :], in_=vm4[:, :, 0, :])

            build_e_chunk(0)
            build_oh8_pair(0)

            pstats = ps.tile([P, NSTAT * NG], F32)

            def process_gather(g, eng):
                # eng: DVE for early gathers; the Pool engine takes the last
                # ones (it sits idle once gather descriptor-gen finishes,
                # while DVE is the backlogged engine gating the PE chain).
                cs = slice(8 * g, 8 * (g + 1))
                # extract: prod = G * E f32 (in place into E), v = sum_e
                eng.tensor_tensor(
                    out=e3[:, cs, :], in0=g3[:, cs, :], in1=e3[:, cs, :],
                    op=OP.mult,
                )
                # free-axis reduce is DVE-only (Pool reduces partitions)
                nc.vector.tensor_reduce(
                    out=v_t[:, cs], in_=e3[:, cs, :],
                    axis=mybir.AxisListType.X, op=OP.add,
                )
                # value stack: v16; r16 = fp16(RSCALE*(v - up(v16))); v16^2
                nc.scalar.copy(out=vs3[:, cs, 0], in_=v_t[:, cs])
                nc.scalar.mul(out=up_t[:, cs], in_=vs3[:, cs, 0], mul=RSCALE)
                nc.scalar.activation(
                    out=vs3[:, cs, 2], in_=vs3[:, cs, 0], func=AF.Square
                )
                nc.vector.scalar_tensor_tensor(
                    out=vs3[:, cs, 1], in0=v_t[:, cs], scalar=RSCALE,
                    in1=up_t[:, cs], op0=OP.mult, op1=OP.subtract,
                )
                # value planes *= vstack (broadcast over the 16 B-groups)
                eng.tensor_tensor(
                    out=vm4[:, cs, 1:4, :],
                    in0=vm4[:, cs, 1:4, :],
                    in1=vs3[:, cs, :][:, :, :, None].to_broadcast([P, 8, 3, NG]),
                    op=OP.mult,
                )
                # histogram matmuls for these 8 token columns
                for c in range(8 * g, 8 * (g + 1)):
                    j, cc = divmod(c, 8)
                    oh_t, base = oh8s[j]
                    nc.tensor.matmul(
                        out=pstats[:],
                        lhsT=oh_t[:, base + P * cc : base + P * (cc + 1)],
                        rhs=vmflat[:, c, :],
                        start=(c == 0),
                        stop=(c == P - 1),
                    )

            for g in range(NSUB):
                if g % 4 == 1 and g < 12:
                    build_e_chunk(g // 4 + 1)
                if g + 1 < NSUB and oh8s[g + 1] is None:
                    build_oh8_pair(g + 1)
                process_gather(g, nc.vector)

            # first-update mask depends only on the preloaded count buffer:
            # compute it before the collective, off the critical path
            first_t = sb.tile([P, NG], U8, name="first_t")
            nc.vector.tensor_scalar(
                out=first_t[:], in0=c_t[:], scalar1=0, scalar2=None,
                op0=OP.is_equal,
            )

            # ---- fold partials [n | s_hi + s_lo/RSCALE | q] -> [128, 48]
            nc.vector.tensor_copy(out=st[:, 0:16], in_=pstats[:, 0:16])
            nc.vector.tensor_copy(out=st[:, 16:32], in_=pstats[:, 16:32])
            nc.vector.scalar_tensor_tensor(
                out=st[:, 16:32], in0=pstats[:, 32:48], scalar=1.0 / RSCALE,
                in1=st[:, 16:32], op0=OP.mult, op1=OP.add,
            )
            nc.vector.tensor_copy(out=st[:, 32:48], in_=pstats[:, 48:64])

            # ---- all-reduce across the 8 cores
            if USE_RDMA_ALLREDUCE:
                nc.gpsimd.trigger_dma(count=None)
                nc.vector.tensor_copy(out=allg[:, 0:48], in_=st[:])
                a2 = allg[:].rearrange("p (h x) -> p h x", h=2)
                red1 = sb.tile([P, 4 * 48], F32)
                # the remote-sem wait is attached AFTER TileContext exits:
                # the single-core scheduling sim cannot satisfy a semaphore
                # that only remote cores increment (deadlock otherwise)
                rdma_wait_ins = nc.vector.tensor_tensor(
                    out=red1[:], in0=a2[:, 0, :], in1=a2[:, 1, :], op=OP.add
                )
                r2 = red1[:].rearrange("p (h x) -> p h x", h=2)
                red2 = sb.tile([P, 2 * 48], F32)
                nc.vector.tensor_tensor(
                    out=red2[:], in0=r2[:, 0, :], in1=r2[:, 1, :], op=OP.add
                )
                r3 = red2[:].rearrange("p (h x) -> p h x", h=2)
                stg = sb.tile([P, 48], F32)
                nc.vector.tensor_tensor(
                    out=stg[:], in0=r3[:, 0, :], in1=r3[:, 1, :], op=OP.add
                )
            else:
                nc.sync.dma_start(out=cc_in[:, :], in_=st[:])
                nc.gpsimd.collective_compute(
                    "AllReduce",
                    OP.add,
                    replica_groups=[list(range(NCORES))],
                    ins=[cc_in.ap().opt()],
                    outs=[cc_out.ap().opt()],
                )
                stg = sb.tile([P, 48], F32)
                nc.sync.dma_start(out=stg[:], in_=cc_out[:, :])

            # ---- EMA update on [128, 16] tiles (class = p*16 + g)
            n_t = stg[:, 0:16]
            s_t = stg[:, 16:32]
            q_t = stg[:, 32:48]

            _tid = [0]

            def t16f(dtype=F32):
                _tid[0] += 1
                return sb.tile([P, NG], dtype, name=f"t16_{_tid[0]}")

            ns_t, rn_t, bm_t, bv_t = t16f(), t16f(), t16f(), t16f()
            nc.vector.tensor_scalar_max(out=ns_t[:], in0=n_t, scalar1=1.0)
            nc.vector.reciprocal(out=rn_t[:], in_=ns_t[:])
            nc.vector.tensor_tensor(out=bm_t[:], in0=s_t, in1=rn_t[:], op=OP.mult)
            qn_t, bm2_t = t16f(), t16f()
            nc.vector.tensor_tensor(out=qn_t[:], in0=q_t, in1=rn_t[:], op=OP.mult)
            nc.vector.tensor_tensor(out=bm2_t[:], in0=bm_t[:], in1=bm_t[:], op=OP.mult)
            nc.vector.tensor_tensor(
                out=bv_t[:], in0=qn_t[:], in1=bm2_t[:], op=OP.subtract
            )

            has_t = t16f(U8)
            nc.vector.tensor_scalar(
                out=has_t[:], in0=n_t, scalar1=0.0, scalar2=None, op0=OP.is_gt
            )

            d_t, em_t, ev_t = t16f(), t16f(), t16f()
            nc.vector.tensor_tensor(out=d_t[:], in0=bm_t[:], in1=m_t[:], op=OP.subtract)
            nc.vector.scalar_tensor_tensor(
                out=em_t[:], in0=d_t[:], scalar=EMA_DECAY, in1=m_t[:],
                op0=OP.mult, op1=OP.add,
            )
            nc.vector.tensor_tensor(
                out=d_t[:], in0=bv_t[:], in1=va_t[:], op=OP.subtract
            )
            nc.vector.scalar_tensor_tensor(
                out=ev_t[:], in0=d_t[:], scalar=EMA_DECAY, in1=va_t[:],
                op0=OP.mult, op1=OP.add,
            )

            cm_t, cv_t = t16f(), t16f()
            nc.vector.select(out=cm_t[:], mask=first_t[:], on_true=bm_t[:], on_false=em_t[:])
            nc.vector.select(out=cv_t[:], mask=first_t[:], on_true=bv_t[:], on_false=ev_t[:])
            nc.vector.tensor_scalar_max(out=cv_t[:], in0=cv_t[:], scalar1=EPS)

            nm_t, nv_t = t16f(), t16f()
            nc.vector.select(out=nm_t[:], mask=has_t[:], on_true=cm_t[:], on_false=m_t[:])
            nc.vector.select(out=nv_t[:], mask=has_t[:], on_true=cv_t[:], on_false=va_t[:])
            ni_t, ncnt_t = t16f(I32), t16f(I32)
            nc.vector.tensor_copy(out=ni_t[:], in_=n_t)
            nc.vector.tensor_tensor(out=ncnt_t[:], in0=c_t[:], in1=ni_t[:], op=OP.add)

            nc.sync.dma_start(
                out=new_mean[:].rearrange("(p c) -> p c", p=P), in_=nm_t[:]
            )
            nc.scalar.dma_start(
                out=new_var[:].rearrange("(p c) -> p c", p=P), in_=nv_t[:]
            )
            nc.sync.dma_start(
                out=new_count[:].rearrange("(p c) -> p c", p=P), in_=ncnt_t[:]
            )

    if USE_RDMA_ALLREDUCE:
        # Post-scheduling: require all 7 peers' partials to have landed
        # (each peer's broadcast bumps our remote sem by 16/8 = 2).
        rdma_wait_ins.wait_op(
            rsem, (NCORES - 1) * (16 // 8), "sem-ge", check=False
        )

    nc.compile()
    return nc


def make_in_maps(logits, target, mean, var, count):
    """Shard the full inputs into per-core input maps."""
    logits = np.ascontiguousarray(np.asarray(logits, dtype=np.float32))
    target = np.asarray(target).astype(np.int32)
    mean = np.asarray(mean, dtype=np.float32)
    var = np.asarray(var, dtype=np.float32)
    count_i32 = np.asarray(count).astype(np.int32)

    in_maps = []
    for m in range(NCORES):
        rows = slice(m * BS, (m + 1) * BS)
        in_maps.append(
            {
                "logits": logits[rows].reshape(BS * K, 1),
                "target": target[rows],
                "mean": mean,
                "var": var,
                "count": count_i32,
            }
        )
    return in_maps


_NC_CACHE = None


def kernel(logits, target, mean, var, count):
    global _NC_CACHE
    if _NC_CACHE is None:
        _NC_CACHE = build_program()
    nc = _NC_CACHE

    in_maps = make_in_maps(logits, target, mean, var, count)
    res = run_bass_kernel_spmd(nc, in_maps, list(range(NCORES)))
    out = res.results[0]

    count_dtype = np.asarray(count).dtype
    return (
        out["new_mean"].reshape(K).astype(np.float32),
        out["new_var"].reshape(K).astype(np.float32),
        out["new_count"].reshape(K).astype(count_dtype),
    )
